# revision 17
# baseline (speedup 1.0000x reference)
"""GRACE contrastive loss on 8 Trainium2 NeuronCores (Bass/Tile).

loss = mean over i of 0.5*(l1_i + l2_i), where (T=0.5, a/b = row-normalized
h1/h2):
  l1_i = log(sum_j exp(a_i.a_j/T) - e^2 + sum_j exp(a_i.b_j/T)) - a_i.b_i/T
  l2_i = log(sum_j exp(b_i.b_j/T) - e^2 + sum_j exp(b_i.a_j/T)) - a_i.b_i/T

Work split over 8 cores, exploiting symmetry of the two reflexive
similarity matrices (only the upper/lower triangle of a@a.T / b@b.T is
exponentiated; the mirrored half is recovered from column sums):

- Phase B (all cores): rows c*1024..(c+1)*1024 of exp(a@b.T): matmul +
  exp with fused row-sum accumulation (ScalarE accum_out), exp values
  staged to SBUF in fp8 for column sums.
- Phase U (all cores): 9 "units" of 1024x1024. In the concatenated
  column-block space [a blocks 0-7 | b blocks 8-15], core c computes
  blocks c..c+8: that is rows a_c x upper-triangle columns of a, plus
  rows b_c x lower-triangle columns of b — a contiguous block run, so a
  single partition-id-derived register offset makes the program SPMD-
  uniform. Each unit emits row sums (accum_out) and column sums (VectorE
  tree-add over row tiles + ones-vector matmul partition reduce on PE).
- cs_ab groups: column sums of exp(a@b.T) via ones-matmuls over the fp8
  staging, PSUM-accumulated across row tiles, interleaved into phase U.

The host does the O(N*D) pieces: normalization, diag(a@b.T), final
assembly of row/column sums into the two denominators, log, mean.
"""

import hashlib
import inspect
import os
import pickle
import types
from contextlib import ExitStack
from pathlib import Path

import ml_dtypes
import numpy as np

TEMPERATURE = 0.5
EPS = 1e-8
N, D = 8192, 128
NCORES = 8
BLK = N // NCORES          # 1024 rows per core / unit side
RT = BLK // 128            # 8 row tiles per block
NU = 9                     # units per core in phase U


def _install_neff_disk_cache():
    """Cache walrus NEFF compiles on disk so fresh-process runs are fast."""
    import concourse.bass2jax as bass2jax

    if getattr(bass2jax, "_grace_neff_cache", False):
        return
    inner = bass2jax.compile_bir_kernel
    cache_dir = Path(os.environ.get("XDG_CACHE_HOME", os.path.expanduser("~/.cache")))
    cache_dir = cache_dir / "bass_neff_cache"
    try:
        cache_dir.mkdir(parents=True, exist_ok=True)
    except OSError:
        return

    def cached(bir_json, tmpdir, neff_name="file.neff"):
        data = bir_json if isinstance(bir_json, bytes) else bir_json.encode()
        key = hashlib.sha256(data).hexdigest()
        path = cache_dir / f"{key}_{neff_name}"
        out_path = os.path.join(tmpdir, neff_name)
        if path.exists():
            with open(path, "rb") as f:
                neff = f.read()
            with open(out_path, "wb") as f:
                f.write(neff)
            return out_path
        res = inner(bir_json, tmpdir, neff_name)
        try:
            with open(res, "rb") as f:
                neff = f.read()
            tmp = path.with_suffix(".tmp%d" % os.getpid())
            with open(tmp, "wb") as f:
                f.write(neff)
            tmp.rename(path)
        except OSError:
            pass
        return res

    bass2jax.compile_bir_kernel = cached
    bass2jax._grace_neff_cache = True


_PROGRAM = None


def build_program():
    global _PROGRAM
    if _PROGRAM is not None:
        return _PROGRAM

    import concourse.bass as bass
    import concourse.tile as tile
    from concourse import bacc, mybir

    BF = mybir.dt.bfloat16
    F8 = mybir.dt.float8e4
    F32 = mybir.dt.float32
    Exp = mybir.ActivationFunctionType.Exp
    X = mybir.AxisListType.X

    nc = bacc.Bacc(
        "TRN2",
        target_bir_lowering=False,
        debug=False,
        enable_asserts=False,
        num_devices=NCORES,
    )
    at_d = nc.dram_tensor("at", [128, N], BF, kind="ExternalInput").ap()
    bt_d = nc.dram_tensor("bt", [128, N], BF, kind="ExternalInput").ap()
    rs_ab_d = nc.dram_tensor("rs_ab", [128, RT], F32, kind="ExternalOutput").ap()
    rs9_d = nc.dram_tensor("rs9", [128, NU * RT], F32, kind="ExternalOutput").ap()
    cs_ab_d = nc.dram_tensor("cs_ab", [1, N], BF, kind="ExternalOutput").ap()
    cs9_d = nc.dram_tensor("cs9", [1, NU * BLK], BF, kind="ExternalOutput").ap()

    with tile.TileContext(nc) as tc, ExitStack() as ctx:
        inp = ctx.enter_context(tc.tile_pool(name="inp", bufs=1))
        expp = ctx.enter_context(tc.tile_pool(name="expst", bufs=1))
        ustp = ctx.enter_context(tc.tile_pool(name="ust", bufs=2))
        lhsp = ctx.enter_context(tc.tile_pool(name="lhst", bufs=2))
        accp = ctx.enter_context(tc.tile_pool(name="acc", bufs=4))
        rsp = ctx.enter_context(tc.tile_pool(name="rs", bufs=1))
        csbp = ctx.enter_context(tc.tile_pool(name="csb", bufs=1))
        onep = ctx.enter_context(tc.tile_pool(name="ones", bufs=1))

        # ---- input DMAs (first-use order) ----
        pid0 = nc.partition_id()
        PIECE = N // 4
        # this core's row blocks, sliced out of the full at/bt by partition id
        lhscat = inp.tile([128, 2 * BLK], BF)          # [a_blk | b_blk] transposed
        nc.sync.dma_start(lhscat[:, 0:BLK], at_d[:, bass.ds(pid0 * BLK, BLK)])
        nc.sync.dma_start(lhscat[:, BLK : 2 * BLK], bt_d[:, bass.ds(pid0 * BLK, BLK)])
        bt_p = []
        for i in range(4):
            t = inp.tile([128, PIECE], BF, tag=f"bt{i}")
            nc.sync.dma_start(t[:], bt_d[:, i * PIECE : (i + 1) * PIECE])
            bt_p.append(t)
        # concatenated [at | bt] column-block space for phase U
        atbt = inp.tile([128, 2 * N], BF)
        nc.sync.dma_start(atbt[:, 0:N], at_d[:])
        nc.sync.dma_start(atbt[:, N : 2 * N], bt_d[:])

        ones8 = onep.tile([128, 1], F8, tag="ones8")
        nc.vector.memset(ones8[:], 1.0)
        ones16 = onep.tile([128, 1], BF, tag="ones16")
        nc.vector.memset(ones16[:], 1.0)

        # fp8 staging of exp(a_blk@b^T) for the cs_ab column sums
        expst = expp.tile([128, RT * N], F8)
        cs_sb = csbp.tile([1, N], BF, tag="cs_sb")
        cs9_sb = csbp.tile([1, NU * BLK], BF, tag="cs9_sb")
        rs9_t = rsp.tile([128, NU * RT], F32, tag="rs9")

        pid = pid0

        # ---- Phase B: between slab, full width, 2048-column ACT chunks ----
        with tc.tile_pool(name="mmB", bufs=2, space="PSUM") as mmB:
            rs_t = rsp.tile([128, RT], F32, tag="rs_ab")
            for rt in range(RT):
                lhsT = lhscat[:, rt * 128 : (rt + 1) * 128]
                acc = accp.tile([128, 4], F32)
                for ci in range(4):
                    mt = mmB.tile([128, 2048], F32)
                    for q in range(4):
                        off = ci * 2048 + q * 512
                        nc.tensor.matmul(
                            mt[:, q * 512 : (q + 1) * 512],
                            lhsT=lhsT,
                            rhs=bt_p[off // PIECE][:, off % PIECE : off % PIECE + 512],
                            start=True,
                            stop=True,
                        )
                    nc.scalar.activation(
                        expst[:, rt * N + ci * 2048 : rt * N + (ci + 1) * 2048],
                        mt[:, :],
                        Exp,
                        scale=2.0,
                        accum_out=acc[:, ci : ci + 1],
                    )
                nc.vector.reduce_sum(rs_t[:, rt : rt + 1], acc[:, :], axis=X)
            nc.sync.dma_start(rs_ab_d[:], rs_t[:])

        # ---- Phase U: 9 symmetric units + interleaved cs_ab groups ----
        with (
            tc.tile_pool(name="mmU", bufs=2, space="PSUM") as mmU,
            tc.tile_pool(name="ucs", bufs=2, space="PSUM") as ucs,
            tc.tile_pool(name="csp", bufs=2, space="PSUM") as csp,
        ):

            def csab_group(ct):
                cst = csp.tile([1, 512], F32)
                for rt in range(RT):
                    nc.tensor.matmul(
                        cst[:, :],
                        lhsT=ones8[:, :],
                        rhs=expst[:, rt * N + ct * 512 : rt * N + (ct + 1) * 512],
                        start=(rt == 0),
                        stop=(rt == RT - 1),
                    )
                nc.vector.tensor_copy(cs_sb[:, ct * 512 : (ct + 1) * 512], cst[:, :])

            csab_sched = iter(range(N // 512))
            for u in range(NU):
                # unit's column block in [at|bt] space: t = pid + u
                base = (pid + u) * BLK
                lhsoff = ((pid + u) & 8) * 128  # 0 -> abt half, 1024 -> bbt half
                lhst = lhsp.tile([128, BLK], BF)
                nc.vector.tensor_copy(lhst[:, :], lhscat[:, bass.ds(lhsoff, BLK)])
                ust = ustp.tile([128, RT * BLK], BF)
                for rt in range(RT):
                    mt = mmU.tile([128, BLK], F32)
                    for q in range(2):
                        nc.tensor.matmul(
                            mt[:, q * 512 : (q + 1) * 512],
                            lhsT=lhst[:, rt * 128 : (rt + 1) * 128],
                            rhs=atbt[:, bass.ds(base + q * 512, 512)],
                            start=True,
                            stop=True,
                        )
                    nc.scalar.activation(
                        ust[:, rt * BLK : (rt + 1) * BLK],
                        mt[:, :],
                        Exp,
                        scale=2.0,
                        accum_out=rs9_t[:, u * RT + rt : u * RT + rt + 1],
                    )
                # column sums of this unit: tree-add the 8 row tiles on DVE,
                # then partition-reduce via ones-matmuls on PE.
                for dst, src in [(1, 0), (3, 2), (5, 4), (7, 6), (3, 1), (7, 5), (7, 3)]:
                    nc.vector.tensor_add(
                        ust[:, dst * BLK : (dst + 1) * BLK],
                        ust[:, dst * BLK : (dst + 1) * BLK],
                        ust[:, src * BLK : (src + 1) * BLK],
                    )
                for h in range(2):
                    uc = ucs.tile([1, 512], F32)
                    nc.tensor.matmul(
                        uc[:, :],
                        lhsT=ones16[:, :],
                        rhs=ust[:, 7 * BLK + h * 512 : 7 * BLK + (h + 1) * 512],
                        start=True,
                        stop=True,
                    )
                    nc.vector.tensor_copy(
                        cs9_sb[:, u * BLK + h * 512 : u * BLK + (h + 1) * 512], uc[:, :]
                    )
                # interleave ~2 cs_ab groups per unit
                for _ in range(2):
                    ct = next(csab_sched, None)
                    if ct is not None:
                        csab_group(ct)
            for ct in csab_sched:
                csab_group(ct)

        nc.sync.dma_start(rs9_d[:], rs9_t[:])
        nc.sync.dma_start(cs9_d[:], cs9_sb[:])
        nc.sync.dma_start(cs_ab_d[:], cs_sb[:])

    nc.compile()
    _PROGRAM = nc
    return nc


def _cache_root():
    d = Path(os.environ.get("XDG_CACHE_HOME", os.path.expanduser("~/.cache")))
    return d / "bass_neff_cache"


_META = None


def _get_program_meta():
    """BIR bytes + IO metadata for the program; builds the Bass program only
    on (disk-)cache miss, so warm processes skip the ~1s bass/Tile build."""
    global _META
    if _META is not None:
        return _META
    key = hashlib.sha256(inspect.getsource(build_program).encode()).hexdigest()[:24]
    path = _cache_root() / f"grace_prog_{key}.pkl"
    if path.exists():
        try:
            with open(path, "rb") as f:
                _META = pickle.load(f)
            return _META
        except Exception:
            pass
    nc = build_program()
    from concourse import mybir

    pname = nc.partition_id_tensor.name if nc.partition_id_tensor else None
    ins, outs = [], []
    for alloc in nc.m.functions[0].allocations:
        if not isinstance(alloc, mybir.MemoryLocationSet):
            continue
        name = alloc.memorylocations[0].name
        if alloc.kind == "ExternalInput":
            if name != pname:
                ins.append(name)
        elif alloc.kind == "ExternalOutput":
            outs.append(
                (
                    name,
                    tuple(alloc.tensor_shape),
                    np.dtype(mybir.dt.np(alloc.dtype)).str,
                )
            )
    _META = {
        "bir": nc.to_json_bytes(),
        "arch": nc.m.arch,
        "ins": ins,
        "outs": outs,
        "pname": pname,
    }
    try:
        path.parent.mkdir(parents=True, exist_ok=True)
        tmp = path.with_suffix(".tmp%d" % os.getpid())
        with open(tmp, "wb") as f:
            pickle.dump(_META, f)
        tmp.rename(path)
    except OSError:
        pass
    return _META


class _NcShim:
    """Duck-typed stand-in for the Bass object in _bass_exec_p lowering."""

    def __init__(self, meta):
        self._bir = meta["bir"]
        self.m = types.SimpleNamespace(arch=meta["arch"])
        self.target_bir_lowering = False
        self.has_collectives = False
        self.dbg_addr = None
        self.dbg_callbacks = ()

    def to_json_bytes(self):
        return self._bir

    def is_finalized(self):
        return True


_JITTED = None


def _run(meta, at, bt):
    """Run the program on 8 cores: at/bt replicated (uploaded once), outputs
    sharded per core. Returns {name: array[NCORES, *shape]}."""
    global _JITTED
    import jax
    import concourse.bass2jax as b2j
    from jax.experimental.shard_map import shard_map
    from jax.sharding import Mesh, PartitionSpec

    out_names = [n for n, _, _ in meta["outs"]]
    if _JITTED is None:
        b2j.install_neuronx_cc_hook()
        shim = _NcShim(meta)
        out_avals = tuple(
            jax.core.ShapedArray(s, np.dtype(d)) for _, s, d in meta["outs"]
        )
        in_names = tuple(meta["ins"]) + tuple(out_names)
        if meta["pname"]:
            in_names = in_names + (meta["pname"],)
        n_params = len(meta["ins"])
        n_outs = len(out_names)

        def _body(*args):
            operands = list(args)
            if meta["pname"]:
                operands.append(b2j.partition_id_tensor())
            outs = b2j._bass_exec_p.bind(
                *operands,
                out_avals=out_avals,
                in_names=in_names,
                out_names=tuple(out_names),
                lowering_input_output_aliases=(),
                sim_require_finite=True,
                sim_require_nnan=True,
                nc=shim,
            )
            return tuple(outs)

        devices = jax.devices()[:NCORES]
        mesh = Mesh(np.asarray(devices), ("core",))
        in_specs = (PartitionSpec(),) * n_params + (PartitionSpec("core"),) * n_outs
        out_specs = (PartitionSpec("core"),) * n_outs
        _JITTED = jax.jit(
            shard_map(
                _body, mesh=mesh, in_specs=in_specs, out_specs=out_specs, check_rep=False
            ),
            donate_argnums=tuple(range(n_params, n_params + n_outs)),
            keep_unused=True,
        )
    zeros = [
        np.zeros((NCORES * s[0], *s[1:]), np.dtype(d)) for _, s, d in meta["outs"]
    ]
    outs = _JITTED(at, bt, *zeros)
    return {
        n: np.asarray(o).reshape(NCORES, *spec[1])
        for n, o, spec in zip(out_names, outs, meta["outs"])
    }


def _normalize(x):
    n = np.linalg.norm(x, axis=1, keepdims=True)
    return x / np.maximum(n, EPS)


def kernel(h1: np.ndarray, h2: np.ndarray):
    h1 = np.asarray(h1, dtype=np.float32)
    h2 = np.asarray(h2, dtype=np.float32)
    assert h1.shape == (N, D) and h2.shape == (N, D)

    a = _normalize(h1)
    b = _normalize(h2)
    diag = np.einsum("ij,ij->i", a, b, dtype=np.float64)

    bf = ml_dtypes.bfloat16
    at = np.ascontiguousarray(a.T).astype(bf)   # [128, 8192]
    bt = np.ascontiguousarray(b.T).astype(bf)

    _install_neff_disk_cache()
    try:
        results = _run(_get_program_meta(), at, bt)
    except Exception:
        # Robust fallback: full build + stock SPMD runner.
        nc = build_program()
        from concourse import bass_utils

        in_maps = [{"at": at, "bt": bt} for _ in range(NCORES)]
        r = bass_utils.run_bass_kernel_spmd(nc, in_maps, core_ids=list(range(NCORES)))
        results = {
            name: np.stack([r.results[c][name] for c in range(NCORES)])
            for name in ("rs_ab", "rs9", "cs_ab", "cs9")
        }

    # ---- host assembly ----
    # row-tile layout [128, RT] -> rows: global row = rt*128 + p
    def rows_of(arr):  # [128, k*RT] -> [k, BLK]
        k = arr.shape[1] // RT
        return arr.astype(np.float64).T.reshape(k, RT, 128).reshape(k, BLK)

    e2 = np.exp(2.0)
    rs_ab = np.concatenate([rows_of(results["rs_ab"][c])[0] for c in range(NCORES)])
    cs_ab = np.sum(
        [results["cs_ab"][c][0].astype(np.float64) for c in range(NCORES)], axis=0
    )

    rs_aa = np.zeros(N, dtype=np.float64)
    rs_bb = np.zeros(N, dtype=np.float64)
    for c in range(NCORES):
        rs9 = rows_of(results["rs9"][c])          # [NU, BLK] row sums per unit
        cs9 = results["cs9"][c][0].astype(np.float64)  # [NU*BLK] col sums per unit
        for u in range(NU):
            t = c + u  # column block in [a 0-7 | b 8-15] space
            if t < NCORES:
                # unit of a@a.T: rows block c, columns block t (t >= c)
                rs_aa[c * BLK : (c + 1) * BLK] += rs9[u]
                if u > 0:  # mirrored half: contributes to rows block t
                    rs_aa[t * BLK : (t + 1) * BLK] += cs9[u * BLK : (u + 1) * BLK]
            else:
                # unit of b@b.T: rows block c, columns block v (v <= c)
                v = t - NCORES
                rs_bb[c * BLK : (c + 1) * BLK] += rs9[u]
                if v < c:  # mirrored half: contributes to rows block v
                    rs_bb[v * BLK : (v + 1) * BLK] += cs9[u * BLK : (u + 1) * BLK]

    denom1 = rs_aa - e2 + rs_ab
    denom2 = rs_bb - e2 + cs_ab
    l1 = np.log(denom1) - 2.0 * diag
    l2 = np.log(denom2) - 2.0 * diag
    loss = np.mean(0.5 * (l1 + l2))
    return (np.asarray(loss, dtype=np.float32), 1)


# revision 19
# speedup vs baseline: 1.6748x; 1.6748x over previous
"""GRACE contrastive loss on 8 Trainium2 NeuronCores (Bass/Tile).

loss = mean over i of 0.5*(l1_i + l2_i), where (T=0.5, a/b = row-normalized
h1/h2):
  l1_i = log(sum_j exp(a_i.a_j/T) - e^2 + sum_j exp(a_i.b_j/T)) - a_i.b_i/T
  l2_i = log(sum_j exp(b_i.b_j/T) - e^2 + sum_j exp(b_i.a_j/T)) - a_i.b_i/T

Work split over 8 cores, exploiting symmetry of the two reflexive
similarity matrices (only the upper/lower triangle of a@a.T / b@b.T is
exponentiated; the mirrored half is recovered from column sums):

- Phase B (all cores): rows c*1024..(c+1)*1024 of exp(a@b.T): matmul +
  exp with fused row-sum accumulation (ScalarE accum_out), exp values
  staged to SBUF in fp8 for column sums.
- Phase U (all cores): 9 "units" of 1024x1024. In the concatenated
  column-block space [a blocks 0-7 | b blocks 8-15], core c computes
  blocks c..c+8: that is rows a_c x upper-triangle columns of a, plus
  rows b_c x lower-triangle columns of b — a contiguous block run, so a
  single partition-id-derived register offset makes the program SPMD-
  uniform. Each unit emits row sums (accum_out) and column sums (VectorE
  tree-add over row tiles + ones-vector matmul partition reduce on PE).
- cs_ab groups: column sums of exp(a@b.T) via ones-matmuls over the fp8
  staging, PSUM-accumulated across row tiles, interleaved into phase U.

The host does the O(N*D) pieces: normalization, diag(a@b.T), final
assembly of row/column sums into the two denominators, log, mean.
"""

import hashlib
import inspect
import os
import pickle
import types
from contextlib import ExitStack
from pathlib import Path

import ml_dtypes
import numpy as np

TEMPERATURE = 0.5
EPS = 1e-8
N, D = 8192, 128
NCORES = 8
BLK = N // NCORES          # 1024 rows per core / unit side
RT = BLK // 128            # 8 row tiles per block
NU = 9                     # units per core in phase U


def _install_neff_disk_cache():
    """Cache walrus NEFF compiles on disk so fresh-process runs are fast."""
    import concourse.bass2jax as bass2jax

    if getattr(bass2jax, "_grace_neff_cache", False):
        return
    inner = bass2jax.compile_bir_kernel
    cache_dir = Path(os.environ.get("XDG_CACHE_HOME", os.path.expanduser("~/.cache")))
    cache_dir = cache_dir / "bass_neff_cache"
    try:
        cache_dir.mkdir(parents=True, exist_ok=True)
    except OSError:
        return

    def cached(bir_json, tmpdir, neff_name="file.neff"):
        data = bir_json if isinstance(bir_json, bytes) else bir_json.encode()
        key = hashlib.sha256(data).hexdigest()
        path = cache_dir / f"{key}_{neff_name}"
        out_path = os.path.join(tmpdir, neff_name)
        if path.exists():
            with open(path, "rb") as f:
                neff = f.read()
            with open(out_path, "wb") as f:
                f.write(neff)
            return out_path
        res = inner(bir_json, tmpdir, neff_name)
        try:
            with open(res, "rb") as f:
                neff = f.read()
            tmp = path.with_suffix(".tmp%d" % os.getpid())
            with open(tmp, "wb") as f:
                f.write(neff)
            tmp.rename(path)
        except OSError:
            pass
        return res

    bass2jax.compile_bir_kernel = cached
    bass2jax._grace_neff_cache = True


_PROGRAM = None


def build_program():
    global _PROGRAM
    if _PROGRAM is not None:
        return _PROGRAM

    import concourse.bass as bass
    import concourse.tile as tile
    from concourse import bacc, mybir

    BF = mybir.dt.bfloat16
    F8 = mybir.dt.float8e4
    F32 = mybir.dt.float32
    Exp = mybir.ActivationFunctionType.Exp
    X = mybir.AxisListType.X

    nc = bacc.Bacc(
        "TRN2",
        target_bir_lowering=False,
        debug=False,
        enable_asserts=False,
        num_devices=NCORES,
    )
    at_d = nc.dram_tensor("at", [128, N], BF, kind="ExternalInput").ap()
    bt_d = nc.dram_tensor("bt", [128, N], BF, kind="ExternalInput").ap()
    rs_ab_d = nc.dram_tensor("rs_ab", [128, RT], F32, kind="ExternalOutput").ap()
    rs9_d = nc.dram_tensor("rs9", [128, NU * RT], F32, kind="ExternalOutput").ap()
    cs_ab_d = nc.dram_tensor("cs_ab", [1, N], BF, kind="ExternalOutput").ap()
    cs9_d = nc.dram_tensor("cs9", [1, NU * BLK], BF, kind="ExternalOutput").ap()

    with tile.TileContext(nc) as tc, ExitStack() as ctx:
        inp = ctx.enter_context(tc.tile_pool(name="inp", bufs=1))
        expp = ctx.enter_context(tc.tile_pool(name="expst", bufs=1))
        ustp = ctx.enter_context(tc.tile_pool(name="ust", bufs=2))
        lhsp = ctx.enter_context(tc.tile_pool(name="lhst", bufs=2))
        accp = ctx.enter_context(tc.tile_pool(name="acc", bufs=4))
        rsp = ctx.enter_context(tc.tile_pool(name="rs", bufs=1))
        csbp = ctx.enter_context(tc.tile_pool(name="csb", bufs=1))
        onep = ctx.enter_context(tc.tile_pool(name="ones", bufs=1))

        # ---- input DMAs (first-use order) ----
        pid0 = nc.partition_id()
        PIECE = N // 4
        # this core's row blocks, sliced out of the full at/bt by partition id
        lhscat = inp.tile([128, 2 * BLK], BF)          # [a_blk | b_blk] transposed
        nc.sync.dma_start(lhscat[:, 0:BLK], at_d[:, bass.ds(pid0 * BLK, BLK)])
        nc.sync.dma_start(lhscat[:, BLK : 2 * BLK], bt_d[:, bass.ds(pid0 * BLK, BLK)])
        bt_p = []
        for i in range(4):
            t = inp.tile([128, PIECE], BF, tag=f"bt{i}")
            nc.sync.dma_start(t[:], bt_d[:, i * PIECE : (i + 1) * PIECE])
            bt_p.append(t)
        # concatenated [at | bt] column-block space for phase U
        atbt = inp.tile([128, 2 * N], BF)
        nc.sync.dma_start(atbt[:, 0:N], at_d[:])
        nc.sync.dma_start(atbt[:, N : 2 * N], bt_d[:])

        ones8 = onep.tile([128, 1], F8, tag="ones8")
        nc.vector.memset(ones8[:], 1.0)
        ones16 = onep.tile([128, 1], BF, tag="ones16")
        nc.vector.memset(ones16[:], 1.0)

        # fp8 staging of exp(a_blk@b^T) for the cs_ab column sums
        expst = expp.tile([128, RT * N], F8)
        cs_sb = csbp.tile([1, N], BF, tag="cs_sb")
        cs9_sb = csbp.tile([1, NU * BLK], BF, tag="cs9_sb")
        rs9_t = rsp.tile([128, NU * RT], F32, tag="rs9")

        pid = pid0

        # ---- Phase B: between slab, full width, 2048-column ACT chunks ----
        with tc.tile_pool(name="mmB", bufs=2, space="PSUM") as mmB:
            rs_t = rsp.tile([128, RT], F32, tag="rs_ab")
            for rt in range(RT):
                lhsT = lhscat[:, rt * 128 : (rt + 1) * 128]
                acc = accp.tile([128, 4], F32)
                for ci in range(4):
                    mt = mmB.tile([128, 2048], F32)
                    for q in range(4):
                        off = ci * 2048 + q * 512
                        nc.tensor.matmul(
                            mt[:, q * 512 : (q + 1) * 512],
                            lhsT=lhsT,
                            rhs=bt_p[off // PIECE][:, off % PIECE : off % PIECE + 512],
                            start=True,
                            stop=True,
                        )
                    nc.scalar.activation(
                        expst[:, rt * N + ci * 2048 : rt * N + (ci + 1) * 2048],
                        mt[:, :],
                        Exp,
                        scale=2.0,
                        accum_out=acc[:, ci : ci + 1],
                    )
                nc.vector.reduce_sum(rs_t[:, rt : rt + 1], acc[:, :], axis=X)
            nc.sync.dma_start(rs_ab_d[:], rs_t[:])

        # ---- Phase U: 9 symmetric units + interleaved cs_ab groups ----
        with (
            tc.tile_pool(name="mmU", bufs=2, space="PSUM") as mmU,
            tc.tile_pool(name="ucs", bufs=2, space="PSUM") as ucs,
            tc.tile_pool(name="csp", bufs=2, space="PSUM") as csp,
        ):

            def csab_group(ct):
                cst = csp.tile([1, 512], F32)
                for rt in range(RT):
                    nc.tensor.matmul(
                        cst[:, :],
                        lhsT=ones8[:, :],
                        rhs=expst[:, rt * N + ct * 512 : rt * N + (ct + 1) * 512],
                        start=(rt == 0),
                        stop=(rt == RT - 1),
                    )
                nc.vector.tensor_copy(cs_sb[:, ct * 512 : (ct + 1) * 512], cst[:, :])

            csab_sched = iter(range(N // 512))
            for u in range(NU):
                # unit's column block in [at|bt] space: t = pid + u
                base = (pid + u) * BLK
                lhsoff = ((pid + u) & 8) * 128  # 0 -> abt half, 1024 -> bbt half
                lhst = lhsp.tile([128, BLK], BF)
                nc.vector.tensor_copy(lhst[:, :], lhscat[:, bass.ds(lhsoff, BLK)])
                ust = ustp.tile([128, RT * BLK], BF)
                for rt in range(RT):
                    mt = mmU.tile([128, BLK], F32)
                    for q in range(2):
                        nc.tensor.matmul(
                            mt[:, q * 512 : (q + 1) * 512],
                            lhsT=lhst[:, rt * 128 : (rt + 1) * 128],
                            rhs=atbt[:, bass.ds(base + q * 512, 512)],
                            start=True,
                            stop=True,
                        )
                    nc.scalar.activation(
                        ust[:, rt * BLK : (rt + 1) * BLK],
                        mt[:, :],
                        Exp,
                        scale=2.0,
                        accum_out=rs9_t[:, u * RT + rt : u * RT + rt + 1],
                    )
                # column sums of this unit: tree-add the 8 row tiles on DVE,
                # then partition-reduce via ones-matmuls on PE.
                for dst, src in [(1, 0), (3, 2), (5, 4), (7, 6), (3, 1), (7, 5), (7, 3)]:
                    nc.vector.tensor_add(
                        ust[:, dst * BLK : (dst + 1) * BLK],
                        ust[:, dst * BLK : (dst + 1) * BLK],
                        ust[:, src * BLK : (src + 1) * BLK],
                    )
                for h in range(2):
                    uc = ucs.tile([1, 512], F32)
                    nc.tensor.matmul(
                        uc[:, :],
                        lhsT=ones16[:, :],
                        rhs=ust[:, 7 * BLK + h * 512 : 7 * BLK + (h + 1) * 512],
                        start=True,
                        stop=True,
                    )
                    nc.vector.tensor_copy(
                        cs9_sb[:, u * BLK + h * 512 : u * BLK + (h + 1) * 512], uc[:, :]
                    )
                # interleave ~2 cs_ab groups per unit
                for _ in range(2):
                    ct = next(csab_sched, None)
                    if ct is not None:
                        csab_group(ct)
            for ct in csab_sched:
                csab_group(ct)

        nc.sync.dma_start(rs9_d[:], rs9_t[:])
        nc.sync.dma_start(cs9_d[:], cs9_sb[:])
        nc.sync.dma_start(cs_ab_d[:], cs_sb[:])

    nc.compile()
    _PROGRAM = nc
    return nc


def _cache_root():
    d = Path(os.environ.get("XDG_CACHE_HOME", os.path.expanduser("~/.cache")))
    return d / "bass_neff_cache"


_META = None


def _get_program_meta():
    """BIR bytes + IO metadata for the program; builds the Bass program only
    on (disk-)cache miss, so warm processes skip the ~1s bass/Tile build."""
    global _META
    if _META is not None:
        return _META
    key = hashlib.sha256(inspect.getsource(build_program).encode()).hexdigest()[:24]
    path = _cache_root() / f"grace_prog_{key}.pkl"
    if path.exists():
        try:
            with open(path, "rb") as f:
                _META = pickle.load(f)
            return _META
        except Exception:
            pass
    nc = build_program()
    from concourse import mybir

    pname = nc.partition_id_tensor.name if nc.partition_id_tensor else None
    ins, outs = [], []
    for alloc in nc.m.functions[0].allocations:
        if not isinstance(alloc, mybir.MemoryLocationSet):
            continue
        name = alloc.memorylocations[0].name
        if alloc.kind == "ExternalInput":
            if name != pname:
                ins.append(name)
        elif alloc.kind == "ExternalOutput":
            # NOTE: keep the np.dtype object itself — .str is '<V2' for
            # ml_dtypes bfloat16 and does not round-trip.
            outs.append((name, tuple(alloc.tensor_shape), np.dtype(mybir.dt.np(alloc.dtype))))
    _META = {
        "bir": nc.to_json_bytes(),
        "arch": nc.m.arch,
        "ins": ins,
        "outs": outs,
        "pname": pname,
    }
    try:
        path.parent.mkdir(parents=True, exist_ok=True)
        tmp = path.with_suffix(".tmp%d" % os.getpid())
        with open(tmp, "wb") as f:
            pickle.dump(_META, f)
        tmp.rename(path)
    except OSError:
        pass
    return _META


class _NcShim:
    """Duck-typed stand-in for the Bass object in _bass_exec_p lowering."""

    def __init__(self, meta):
        self._bir = meta["bir"]
        self.m = types.SimpleNamespace(arch=meta["arch"])
        self.target_bir_lowering = False
        self.has_collectives = False
        self.dbg_addr = None
        self.dbg_callbacks = ()

    def to_json_bytes(self):
        return self._bir

    def is_finalized(self):
        return True


_JITTED = None


def _run(meta, at, bt):
    """Run the program on 8 cores: at/bt replicated (uploaded once), outputs
    sharded per core. Returns {name: array[NCORES, *shape]}."""
    global _JITTED
    import jax
    import concourse.bass2jax as b2j
    from jax.experimental.shard_map import shard_map
    from jax.sharding import Mesh, PartitionSpec

    out_names = [n for n, _, _ in meta["outs"]]
    if _JITTED is None:
        b2j.install_neuronx_cc_hook()
        shim = _NcShim(meta)
        out_avals = tuple(
            jax.core.ShapedArray(s, np.dtype(d)) for _, s, d in meta["outs"]
        )
        in_names = tuple(meta["ins"]) + tuple(out_names)
        if meta["pname"]:
            in_names = in_names + (meta["pname"],)
        n_params = len(meta["ins"])
        n_outs = len(out_names)

        def _body(*args):
            operands = list(args)
            if meta["pname"]:
                operands.append(b2j.partition_id_tensor())
            outs = b2j._bass_exec_p.bind(
                *operands,
                out_avals=out_avals,
                in_names=in_names,
                out_names=tuple(out_names),
                lowering_input_output_aliases=(),
                sim_require_finite=True,
                sim_require_nnan=True,
                nc=shim,
            )
            return tuple(outs)

        devices = jax.devices()[:NCORES]
        mesh = Mesh(np.asarray(devices), ("core",))
        in_specs = (PartitionSpec(),) * n_params + (PartitionSpec("core"),) * n_outs
        out_specs = (PartitionSpec("core"),) * n_outs
        _JITTED = jax.jit(
            shard_map(
                _body, mesh=mesh, in_specs=in_specs, out_specs=out_specs, check_rep=False
            ),
            donate_argnums=tuple(range(n_params, n_params + n_outs)),
            keep_unused=True,
        )
    zeros = [
        np.zeros((NCORES * s[0], *s[1:]), np.dtype(d)) for _, s, d in meta["outs"]
    ]
    outs = _JITTED(at, bt, *zeros)
    return {
        n: np.asarray(o).reshape(NCORES, *spec[1])
        for n, o, spec in zip(out_names, outs, meta["outs"])
    }


def _normalize(x):
    n = np.linalg.norm(x, axis=1, keepdims=True)
    return x / np.maximum(n, EPS)


def kernel(h1: np.ndarray, h2: np.ndarray):
    h1 = np.asarray(h1, dtype=np.float32)
    h2 = np.asarray(h2, dtype=np.float32)
    assert h1.shape == (N, D) and h2.shape == (N, D)

    a = _normalize(h1)
    b = _normalize(h2)
    diag = np.einsum("ij,ij->i", a, b, dtype=np.float64)

    bf = ml_dtypes.bfloat16
    at = np.ascontiguousarray(a.T).astype(bf)   # [128, 8192]
    bt = np.ascontiguousarray(b.T).astype(bf)

    _install_neff_disk_cache()
    try:
        results = _run(_get_program_meta(), at, bt)
    except Exception as e:
        import traceback

        print(f"grace fast path failed ({e!r}); falling back", flush=True)
        traceback.print_exc()
        # Robust fallback: full build + stock SPMD runner.
        nc = build_program()
        from concourse import bass_utils

        in_maps = [{"at": at, "bt": bt} for _ in range(NCORES)]
        r = bass_utils.run_bass_kernel_spmd(nc, in_maps, core_ids=list(range(NCORES)))
        results = {
            name: np.stack([r.results[c][name] for c in range(NCORES)])
            for name in ("rs_ab", "rs9", "cs_ab", "cs9")
        }

    # ---- host assembly ----
    # row-tile layout [128, RT] -> rows: global row = rt*128 + p
    def rows_of(arr):  # [128, k*RT] -> [k, BLK]
        k = arr.shape[1] // RT
        return arr.astype(np.float64).T.reshape(k, RT, 128).reshape(k, BLK)

    e2 = np.exp(2.0)
    rs_ab = np.concatenate([rows_of(results["rs_ab"][c])[0] for c in range(NCORES)])
    cs_ab = np.sum(
        [results["cs_ab"][c][0].astype(np.float64) for c in range(NCORES)], axis=0
    )

    rs_aa = np.zeros(N, dtype=np.float64)
    rs_bb = np.zeros(N, dtype=np.float64)
    for c in range(NCORES):
        rs9 = rows_of(results["rs9"][c])          # [NU, BLK] row sums per unit
        cs9 = results["cs9"][c][0].astype(np.float64)  # [NU*BLK] col sums per unit
        for u in range(NU):
            t = c + u  # column block in [a 0-7 | b 8-15] space
            if t < NCORES:
                # unit of a@a.T: rows block c, columns block t (t >= c)
                rs_aa[c * BLK : (c + 1) * BLK] += rs9[u]
                if u > 0:  # mirrored half: contributes to rows block t
                    rs_aa[t * BLK : (t + 1) * BLK] += cs9[u * BLK : (u + 1) * BLK]
            else:
                # unit of b@b.T: rows block c, columns block v (v <= c)
                v = t - NCORES
                rs_bb[c * BLK : (c + 1) * BLK] += rs9[u]
                if v < c:  # mirrored half: contributes to rows block v
                    rs_bb[v * BLK : (v + 1) * BLK] += cs9[u * BLK : (u + 1) * BLK]

    denom1 = rs_aa - e2 + rs_ab
    denom2 = rs_bb - e2 + cs_ab
    l1 = np.log(denom1) - 2.0 * diag
    l2 = np.log(denom2) - 2.0 * diag
    loss = np.mean(0.5 * (l1 + l2))
    return (np.asarray(loss, dtype=np.float32), 1)


# revision 20
# speedup vs baseline: 1.8127x; 1.0823x over previous
"""GRACE contrastive loss on 8 Trainium2 NeuronCores (Bass/Tile).

loss = mean over i of 0.5*(l1_i + l2_i), where (T=0.5, a/b = row-normalized
h1/h2):
  l1_i = log(sum_j exp(a_i.a_j/T) - e^2 + sum_j exp(a_i.b_j/T)) - a_i.b_i/T
  l2_i = log(sum_j exp(b_i.b_j/T) - e^2 + sum_j exp(b_i.a_j/T)) - a_i.b_i/T

Work split over 8 cores, exploiting symmetry of the two reflexive
similarity matrices (only the upper/lower triangle of a@a.T / b@b.T is
exponentiated; the mirrored half is recovered from column sums):

- Phase B (all cores): rows c*1024..(c+1)*1024 of exp(a@b.T): matmul +
  exp with fused row-sum accumulation (ScalarE accum_out), exp values
  staged to SBUF in fp8 for column sums.
- Phase U (all cores): 9 "units" of 1024x1024. In the concatenated
  column-block space [a blocks 0-7 | b blocks 8-15], core c computes
  blocks c..c+8: that is rows a_c x upper-triangle columns of a, plus
  rows b_c x lower-triangle columns of b — a contiguous block run, so a
  single partition-id-derived register offset makes the program SPMD-
  uniform. Each unit emits row sums (accum_out) and column sums (VectorE
  tree-add over row tiles + ones-vector matmul partition reduce on PE).
- cs_ab groups: column sums of exp(a@b.T) via ones-matmuls over the fp8
  staging, PSUM-accumulated across row tiles, interleaved into phase U.

The host does the O(N*D) pieces: normalization, diag(a@b.T), final
assembly of row/column sums into the two denominators, log, mean.
"""

import hashlib
import inspect
import os
import pickle
import types
from contextlib import ExitStack
from pathlib import Path

import ml_dtypes
import numpy as np

TEMPERATURE = 0.5
EPS = 1e-8
N, D = 8192, 128
NCORES = 8
BLK = N // NCORES          # 1024 rows per core / unit side
RT = BLK // 128            # 8 row tiles per block
NU = 9                     # units per core in phase U


def _install_neff_disk_cache():
    """Cache walrus NEFF compiles on disk so fresh-process runs are fast."""
    import concourse.bass2jax as bass2jax

    if getattr(bass2jax, "_grace_neff_cache", False):
        return
    inner = bass2jax.compile_bir_kernel
    cache_dir = Path(os.environ.get("XDG_CACHE_HOME", os.path.expanduser("~/.cache")))
    cache_dir = cache_dir / "bass_neff_cache"
    try:
        cache_dir.mkdir(parents=True, exist_ok=True)
    except OSError:
        return

    def cached(bir_json, tmpdir, neff_name="file.neff"):
        data = bir_json if isinstance(bir_json, bytes) else bir_json.encode()
        key = hashlib.sha256(data).hexdigest()
        path = cache_dir / f"{key}_{neff_name}"
        out_path = os.path.join(tmpdir, neff_name)
        if path.exists():
            with open(path, "rb") as f:
                neff = f.read()
            with open(out_path, "wb") as f:
                f.write(neff)
            return out_path
        res = inner(bir_json, tmpdir, neff_name)
        try:
            with open(res, "rb") as f:
                neff = f.read()
            tmp = path.with_suffix(".tmp%d" % os.getpid())
            with open(tmp, "wb") as f:
                f.write(neff)
            tmp.rename(path)
        except OSError:
            pass
        return res

    bass2jax.compile_bir_kernel = cached
    bass2jax._grace_neff_cache = True


_PROGRAM = None


def build_program():
    global _PROGRAM
    if _PROGRAM is not None:
        return _PROGRAM

    import concourse.bass as bass
    import concourse.tile as tile
    from concourse import bacc, mybir

    BF = mybir.dt.bfloat16
    F8 = mybir.dt.float8e4
    F32 = mybir.dt.float32
    Exp = mybir.ActivationFunctionType.Exp
    X = mybir.AxisListType.X

    nc = bacc.Bacc(
        "TRN2",
        target_bir_lowering=False,
        debug=False,
        enable_asserts=False,
        num_devices=NCORES,
    )
    at_d = nc.dram_tensor("at", [128, N], BF, kind="ExternalInput").ap()
    bt_d = nc.dram_tensor("bt", [128, N], BF, kind="ExternalInput").ap()
    rs_ab_d = nc.dram_tensor("rs_ab", [128, RT], F32, kind="ExternalOutput").ap()
    rs9_d = nc.dram_tensor("rs9", [128, NU * RT], F32, kind="ExternalOutput").ap()
    cs_ab_d = nc.dram_tensor("cs_ab", [1, N], BF, kind="ExternalOutput").ap()
    cs9_d = nc.dram_tensor("cs9", [1, NU * BLK], BF, kind="ExternalOutput").ap()

    with tile.TileContext(nc) as tc, ExitStack() as ctx:
        inp = ctx.enter_context(tc.tile_pool(name="inp", bufs=1))
        expp = ctx.enter_context(tc.tile_pool(name="expst", bufs=1))
        ustp = ctx.enter_context(tc.tile_pool(name="ust", bufs=2))
        lhsp = ctx.enter_context(tc.tile_pool(name="lhst", bufs=2))
        accp = ctx.enter_context(tc.tile_pool(name="acc", bufs=4))
        rsp = ctx.enter_context(tc.tile_pool(name="rs", bufs=1))
        csbp = ctx.enter_context(tc.tile_pool(name="csb", bufs=1))
        onep = ctx.enter_context(tc.tile_pool(name="ones", bufs=1))

        # ---- input DMAs (first-use order) ----
        pid0 = nc.partition_id()
        PIECE = N // 4
        # this core's row blocks, sliced out of the full at/bt by partition id
        lhscat = inp.tile([128, 2 * BLK], BF)          # [a_blk | b_blk] transposed
        nc.sync.dma_start(lhscat[:, 0:BLK], at_d[:, bass.ds(pid0 * BLK, BLK)])
        nc.sync.dma_start(lhscat[:, BLK : 2 * BLK], bt_d[:, bass.ds(pid0 * BLK, BLK)])
        bt_p = []
        for i in range(4):
            t = inp.tile([128, PIECE], BF, tag=f"bt{i}")
            nc.sync.dma_start(t[:], bt_d[:, i * PIECE : (i + 1) * PIECE])
            bt_p.append(t)
        # concatenated [at | bt] column-block space for phase U
        atbt = inp.tile([128, 2 * N], BF)
        nc.sync.dma_start(atbt[:, 0:N], at_d[:])
        nc.sync.dma_start(atbt[:, N : 2 * N], bt_d[:])

        ones8 = onep.tile([128, 1], F8, tag="ones8")
        nc.vector.memset(ones8[:], 1.0)
        ones16 = onep.tile([128, 1], BF, tag="ones16")
        nc.vector.memset(ones16[:], 1.0)

        # fp8 staging of exp(a_blk@b^T) for the cs_ab column sums
        expst = expp.tile([128, RT * N], F8)
        cs_sb = csbp.tile([1, N], BF, tag="cs_sb")
        cs9_sb = csbp.tile([1, NU * BLK], BF, tag="cs9_sb")
        rs9_t = rsp.tile([128, NU * RT], F32, tag="rs9")

        pid = pid0

        # ---- Phase B: between slab, full width, 2048-column ACT chunks ----
        with tc.tile_pool(name="mmB", bufs=2, space="PSUM") as mmB:
            rs_t = rsp.tile([128, RT], F32, tag="rs_ab")
            for rt in range(RT):
                lhsT = lhscat[:, rt * 128 : (rt + 1) * 128]
                acc = accp.tile([128, 4], F32)
                for ci in range(4):
                    mt = mmB.tile([128, 2048], F32)
                    for q in range(4):
                        off = ci * 2048 + q * 512
                        nc.tensor.matmul(
                            mt[:, q * 512 : (q + 1) * 512],
                            lhsT=lhsT,
                            rhs=bt_p[off // PIECE][:, off % PIECE : off % PIECE + 512],
                            start=True,
                            stop=True,
                        )
                    nc.scalar.activation(
                        expst[:, rt * N + ci * 2048 : rt * N + (ci + 1) * 2048],
                        mt[:, :],
                        Exp,
                        scale=2.0,
                        accum_out=acc[:, ci : ci + 1],
                    )
                nc.vector.reduce_sum(rs_t[:, rt : rt + 1], acc[:, :], axis=X)
            nc.sync.dma_start(rs_ab_d[:], rs_t[:])

        # ---- Phase U: 9 symmetric units + interleaved cs_ab groups ----
        with (
            tc.tile_pool(name="mmU", bufs=2, space="PSUM") as mmU,
            tc.tile_pool(name="ucs", bufs=2, space="PSUM") as ucs,
            tc.tile_pool(name="csp", bufs=2, space="PSUM") as csp,
        ):

            def csab_group(ct):
                cst = csp.tile([1, 512], F32)
                for rt in range(RT):
                    nc.tensor.matmul(
                        cst[:, :],
                        lhsT=ones8[:, :],
                        rhs=expst[:, rt * N + ct * 512 : rt * N + (ct + 1) * 512],
                        start=(rt == 0),
                        stop=(rt == RT - 1),
                    )
                nc.vector.tensor_copy(cs_sb[:, ct * 512 : (ct + 1) * 512], cst[:, :])

            csab_sched = iter(range(N // 512))
            for u in range(NU):
                # unit's column block in [at|bt] space: t = pid + u
                base = (pid + u) * BLK
                lhsoff = ((pid + u) & 8) * 128  # 0 -> abt half, 1024 -> bbt half
                lhst = lhsp.tile([128, BLK], BF)
                nc.vector.tensor_copy(lhst[:, :], lhscat[:, bass.ds(lhsoff, BLK)])
                ust = ustp.tile([128, RT * BLK], BF)
                for rt in range(RT):
                    mt = mmU.tile([128, BLK], F32)
                    for q in range(2):
                        nc.tensor.matmul(
                            mt[:, q * 512 : (q + 1) * 512],
                            lhsT=lhst[:, rt * 128 : (rt + 1) * 128],
                            rhs=atbt[:, bass.ds(base + q * 512, 512)],
                            start=True,
                            stop=True,
                        )
                    nc.scalar.activation(
                        ust[:, rt * BLK : (rt + 1) * BLK],
                        mt[:, :],
                        Exp,
                        scale=2.0,
                        accum_out=rs9_t[:, u * RT + rt : u * RT + rt + 1],
                    )
                # column sums of this unit: tree-add the 8 row tiles on DVE,
                # then partition-reduce via ones-matmuls on PE.
                for dst, src in [(1, 0), (3, 2), (5, 4), (7, 6), (3, 1), (7, 5), (7, 3)]:
                    nc.vector.tensor_add(
                        ust[:, dst * BLK : (dst + 1) * BLK],
                        ust[:, dst * BLK : (dst + 1) * BLK],
                        ust[:, src * BLK : (src + 1) * BLK],
                    )
                for h in range(2):
                    uc = ucs.tile([1, 512], F32)
                    nc.tensor.matmul(
                        uc[:, :],
                        lhsT=ones16[:, :],
                        rhs=ust[:, 7 * BLK + h * 512 : 7 * BLK + (h + 1) * 512],
                        start=True,
                        stop=True,
                    )
                    nc.vector.tensor_copy(
                        cs9_sb[:, u * BLK + h * 512 : u * BLK + (h + 1) * 512], uc[:, :]
                    )
                # interleave ~2 cs_ab groups per unit
                for _ in range(2):
                    ct = next(csab_sched, None)
                    if ct is not None:
                        csab_group(ct)
            for ct in csab_sched:
                csab_group(ct)

        nc.sync.dma_start(rs9_d[:], rs9_t[:])
        nc.sync.dma_start(cs9_d[:], cs9_sb[:])
        nc.sync.dma_start(cs_ab_d[:], cs_sb[:])

    nc.compile()
    _PROGRAM = nc
    return nc


def _cache_root():
    d = Path(os.environ.get("XDG_CACHE_HOME", os.path.expanduser("~/.cache")))
    return d / "bass_neff_cache"


_META = None


def _get_program_meta():
    """BIR bytes + IO metadata for the program; builds the Bass program only
    on (disk-)cache miss, so warm processes skip the ~1s bass/Tile build."""
    global _META
    if _META is not None:
        return _META
    src = inspect.getsource(build_program) + "|meta_v3"
    key = hashlib.sha256(src.encode()).hexdigest()[:24]
    path = _cache_root() / f"grace_prog_{key}.pkl"
    if path.exists():
        try:
            with open(path, "rb") as f:
                _META = pickle.load(f)
            return _META
        except Exception:
            pass
    nc = build_program()
    from concourse import mybir

    pname = nc.partition_id_tensor.name if nc.partition_id_tensor else None
    ins, outs = [], []
    for alloc in nc.m.functions[0].allocations:
        if not isinstance(alloc, mybir.MemoryLocationSet):
            continue
        name = alloc.memorylocations[0].name
        if alloc.kind == "ExternalInput":
            if name != pname:
                ins.append(name)
        elif alloc.kind == "ExternalOutput":
            # NOTE: keep the np.dtype object itself — .str is '<V2' for
            # ml_dtypes bfloat16 and does not round-trip.
            outs.append((name, tuple(alloc.tensor_shape), np.dtype(mybir.dt.np(alloc.dtype))))
    _META = {
        "bir": nc.to_json_bytes(),
        "arch": nc.m.arch,
        "ins": ins,
        "outs": outs,
        "pname": pname,
    }
    try:
        path.parent.mkdir(parents=True, exist_ok=True)
        tmp = path.with_suffix(".tmp%d" % os.getpid())
        with open(tmp, "wb") as f:
            pickle.dump(_META, f)
        tmp.rename(path)
    except OSError:
        pass
    return _META


class _NcShim:
    """Duck-typed stand-in for the Bass object in _bass_exec_p lowering."""

    def __init__(self, meta):
        self._bir = meta["bir"]
        self.m = types.SimpleNamespace(arch=meta["arch"])
        self.target_bir_lowering = False
        self.has_collectives = False
        self.dbg_addr = None
        self.dbg_callbacks = ()

    def to_json_bytes(self):
        return self._bir

    def is_finalized(self):
        return True


_JITTED = None


def _run(meta, at, bt):
    """Run the program on 8 cores: at/bt replicated (uploaded once), outputs
    sharded per core. Returns {name: array[NCORES, *shape]}."""
    global _JITTED
    import jax
    import concourse.bass2jax as b2j
    from jax.experimental.shard_map import shard_map
    from jax.sharding import Mesh, PartitionSpec

    out_names = [n for n, _, _ in meta["outs"]]
    if _JITTED is None:
        b2j.install_neuronx_cc_hook()
        shim = _NcShim(meta)
        out_avals = tuple(
            jax.core.ShapedArray(s, np.dtype(d)) for _, s, d in meta["outs"]
        )
        in_names = tuple(meta["ins"]) + tuple(out_names)
        if meta["pname"]:
            in_names = in_names + (meta["pname"],)
        n_params = len(meta["ins"])
        n_outs = len(out_names)

        def _body(*args):
            operands = list(args)
            if meta["pname"]:
                operands.append(b2j.partition_id_tensor())
            outs = b2j._bass_exec_p.bind(
                *operands,
                out_avals=out_avals,
                in_names=in_names,
                out_names=tuple(out_names),
                lowering_input_output_aliases=(),
                sim_require_finite=True,
                sim_require_nnan=True,
                nc=shim,
            )
            return tuple(outs)

        devices = jax.devices()[:NCORES]
        mesh = Mesh(np.asarray(devices), ("core",))
        in_specs = (PartitionSpec(),) * n_params + (PartitionSpec("core"),) * n_outs
        out_specs = (PartitionSpec("core"),) * n_outs
        _JITTED = jax.jit(
            shard_map(
                _body, mesh=mesh, in_specs=in_specs, out_specs=out_specs, check_rep=False
            ),
            donate_argnums=tuple(range(n_params, n_params + n_outs)),
            keep_unused=True,
        )
    zeros = [
        np.zeros((NCORES * s[0], *s[1:]), np.dtype(d)) for _, s, d in meta["outs"]
    ]
    outs = _JITTED(at, bt, *zeros)
    return {
        n: np.asarray(o).reshape(NCORES, *spec[1])
        for n, o, spec in zip(out_names, outs, meta["outs"])
    }


def _normalize(x):
    n = np.linalg.norm(x, axis=1, keepdims=True)
    return x / np.maximum(n, EPS)


def kernel(h1: np.ndarray, h2: np.ndarray):
    h1 = np.asarray(h1, dtype=np.float32)
    h2 = np.asarray(h2, dtype=np.float32)
    assert h1.shape == (N, D) and h2.shape == (N, D)

    a = _normalize(h1)
    b = _normalize(h2)
    diag = np.einsum("ij,ij->i", a, b, dtype=np.float64)

    bf = ml_dtypes.bfloat16
    at = np.ascontiguousarray(a.T).astype(bf)   # [128, 8192]
    bt = np.ascontiguousarray(b.T).astype(bf)

    _install_neff_disk_cache()
    try:
        results = _run(_get_program_meta(), at, bt)
    except Exception as e:
        import traceback

        print(f"grace fast path failed ({e!r}); falling back", flush=True)
        traceback.print_exc()
        # Robust fallback: full build + stock SPMD runner.
        nc = build_program()
        from concourse import bass_utils

        in_maps = [{"at": at, "bt": bt} for _ in range(NCORES)]
        r = bass_utils.run_bass_kernel_spmd(nc, in_maps, core_ids=list(range(NCORES)))
        results = {
            name: np.stack([r.results[c][name] for c in range(NCORES)])
            for name in ("rs_ab", "rs9", "cs_ab", "cs9")
        }

    # ---- host assembly ----
    # row-tile layout [128, RT] -> rows: global row = rt*128 + p
    def rows_of(arr):  # [128, k*RT] -> [k, BLK]
        k = arr.shape[1] // RT
        return arr.astype(np.float64).T.reshape(k, RT, 128).reshape(k, BLK)

    e2 = np.exp(2.0)
    rs_ab = np.concatenate([rows_of(results["rs_ab"][c])[0] for c in range(NCORES)])
    cs_ab = np.sum(
        [results["cs_ab"][c][0].astype(np.float64) for c in range(NCORES)], axis=0
    )

    rs_aa = np.zeros(N, dtype=np.float64)
    rs_bb = np.zeros(N, dtype=np.float64)
    for c in range(NCORES):
        rs9 = rows_of(results["rs9"][c])          # [NU, BLK] row sums per unit
        cs9 = results["cs9"][c][0].astype(np.float64)  # [NU*BLK] col sums per unit
        for u in range(NU):
            t = c + u  # column block in [a 0-7 | b 8-15] space
            if t < NCORES:
                # unit of a@a.T: rows block c, columns block t (t >= c)
                rs_aa[c * BLK : (c + 1) * BLK] += rs9[u]
                if u > 0:  # mirrored half: contributes to rows block t
                    rs_aa[t * BLK : (t + 1) * BLK] += cs9[u * BLK : (u + 1) * BLK]
            else:
                # unit of b@b.T: rows block c, columns block v (v <= c)
                v = t - NCORES
                rs_bb[c * BLK : (c + 1) * BLK] += rs9[u]
                if v < c:  # mirrored half: contributes to rows block v
                    rs_bb[v * BLK : (v + 1) * BLK] += cs9[u * BLK : (u + 1) * BLK]

    denom1 = rs_aa - e2 + rs_ab
    denom2 = rs_bb - e2 + cs_ab
    l1 = np.log(denom1) - 2.0 * diag
    l2 = np.log(denom2) - 2.0 * diag
    loss = np.mean(0.5 * (l1 + l2))
    return (np.asarray(loss, dtype=np.float32), 1)
